# revision 4
# baseline (speedup 1.0000x reference)
"""Trainium2 Bass kernel for EpisodicSlotMemoryBlock.

Data-parallel over batch: B=8192 split across 8 NeuronCores (1024 each).
Per core, per 128-sample tile:
  - keys in sample-major SBUF layout -> slot norms + read/write dots via
    ScalarE square + DVE segmented reduces (+ GPSIMD for one big multiply)
  - vals in (s,k)-partition layout -> read_out via 32 tiny block-diagonal
    PE matmuls, written PSUM->DRAM directly
  - softmax / argmax / straight-through one-hot smalls in [128,32] tiles
  - keys_new/vals_new are a dense unchanged write-back plus an
    indirect-DMA scatter of the 1 updated slot row per sample (the
    straight-through estimator makes write_w exactly one-hot in fp32)
"""

import sys

sys.path.insert(0, "/opt/trn_rl_repo")

import numpy as np

import concourse.bacc as bacc
import concourse.bass as bass
import concourse.tile as tile
from concourse import mybir
from concourse.bass import IndirectOffsetOnAxis
from concourse.masks import make_identity

F32 = mybir.dt.float32
I32 = mybir.dt.int32
U32 = mybir.dt.uint32
AF = mybir.ActivationFunctionType
OP = mybir.AluOpType
AX = mybir.AxisListType

P = 128
K = 32
D = 256
KD = K * D
NCORES = 8
B_FULL = 8192
BC = B_FULL // NCORES

STRENGTH_DECAY = 0.995
AGE_PENALTY = 0.02
STRENGTH_BOOST = 0.5
WRITE_ALPHA = 0.5
WRITE_TEMP = 50.0
EVICT_AGE_BOOST = 0.05
EVICT_STR_PENALTY = 0.5
EPS = 1e-6


def build(bc=BC):
    nc = bacc.Bacc("TRN2", target_bir_lowering=False, debug=False)
    nt = bc // P

    q_d = nc.dram_tensor("query_vec", [bc, D], F32, kind="ExternalInput")
    wv_d = nc.dram_tensor("write_vec", [bc, D], F32, kind="ExternalInput")
    ws_d = nc.dram_tensor("write_strength", [bc, 1], F32, kind="ExternalInput")
    keys_d = nc.dram_tensor("keys", [bc, K, D], F32, kind="ExternalInput")
    vals_d = nc.dram_tensor("vals", [bc, K, D], F32, kind="ExternalInput")
    age_d = nc.dram_tensor("age", [bc, K], F32, kind="ExternalInput")
    str_d = nc.dram_tensor("strength", [bc, K], F32, kind="ExternalInput")

    ro_d = nc.dram_tensor("read_out", [bc, D], F32, kind="ExternalOutput")
    kn_d = nc.dram_tensor("keys_new", [bc, K, D], F32, kind="ExternalOutput")
    vn_d = nc.dram_tensor("vals_new", [bc, K, D], F32, kind="ExternalOutput")
    an_d = nc.dram_tensor("age_new", [bc, K], F32, kind="ExternalOutput")
    sn_d = nc.dram_tensor("str_new", [bc, K], F32, kind="ExternalOutput")

    keys_flat = keys_d[:].rearrange("b k d -> (b k) d")
    vals_flat = vals_d[:].rearrange("b k d -> (b k) d")
    kn_flat = kn_d[:].rearrange("b k d -> (b k) d")
    vn_flat = vn_d[:].rearrange("b k d -> (b k) d")

    KH = K // 2  # 16 slots per keys half
    GH = K // 2  # 16 groups per vals half

    with tile.TileContext(nc) as tc:
        with (
            tc.tile_pool(name="keysp", bufs=2) as keysp,
            tc.tile_pool(name="valsp", bufs=2) as valsp,
            tc.tile_pool(name="scrp", bufs=3) as scrp,
            tc.tile_pool(name="smallp", bufs=2) as smallp,
            tc.tile_pool(name="rowp", bufs=2) as rowp,
            tc.tile_pool(name="persist", bufs=1) as persist,
            tc.tile_pool(name="diagp", bufs=3) as diagp,
            tc.tile_pool(name="psump", bufs=2, space="PSUM") as psump,
        ):
            # persistent across tiles: scatter rows + indices
            rows_k = persist.tile([P, nt * D], F32)
            rows_v = persist.tile([P, nt * D], F32)
            gidx_all = persist.tile([P, nt], I32)
            ident = persist.tile([P, P], F32)
            make_identity(nc, ident[:])

            for t in range(nt):
                b0 = t * P

                # ---------------- loads ----------------
                keys_h = []
                for h in range(2):
                    kh = keysp.tile([P, KH * D], F32, tag="keysh")
                    nc.sync.dma_start(
                        kh[:].rearrange("p (k d) -> p k d", k=KH),
                        keys_d[b0 : b0 + P, h * KH : (h + 1) * KH, :],
                    )
                    keys_h.append(kh)

                vals_h = []
                for h in range(2):
                    vh = valsp.tile([P, KH * D], F32, tag="valsh")
                    nc.sync.dma_start(
                        vh[:].rearrange("p (k d) -> p k d", k=KH),
                        vals_d[b0 : b0 + P, h * KH : (h + 1) * KH, :],
                    )
                    vals_h.append(vh)

                q_t = smallp.tile([P, D], F32, tag="q")
                nc.sync.dma_start(q_t[:], q_d[b0 : b0 + P, :])
                wv_t = smallp.tile([P, D], F32, tag="wv")
                nc.sync.dma_start(wv_t[:], wv_d[b0 : b0 + P, :])
                ws_t = smallp.tile([P, 1], F32, tag="ws")
                nc.sync.dma_start(ws_t[:], ws_d[b0 : b0 + P, :])
                age_t = smallp.tile([P, K], F32, tag="age")
                nc.sync.dma_start(age_t[:], age_d[b0 : b0 + P, :])
                str_t = smallp.tile([P, K], F32, tag="str")
                nc.sync.dma_start(str_t[:], str_d[b0 : b0 + P, :])

                # ---------------- big stats ----------------
                normsq = smallp.tile([P, K], F32, tag="normsq")
                dot_q = smallp.tile([P, K], F32, tag="dot_q")
                dot_w = smallp.tile([P, K], F32, tag="dot_w")
                for h in range(2):
                    kh = keys_h[h]
                    kh3 = kh[:].rearrange("p (k d) -> p k d", k=KH)
                    ks = slice(h * KH, (h + 1) * KH)

                    sq = scrp.tile([P, KH * D], F32, tag="scr")
                    nc.scalar.activation(
                        sq[:], kh[:], AF.Square
                    )
                    nc.vector.tensor_reduce(
                        normsq[:, ks],
                        sq[:].rearrange("p (k d) -> p k d", k=KH),
                        axis=AX.X,
                        op=OP.add,
                    )

                    pq = scrp.tile([P, KH * D], F32, tag="scr")
                    nc.vector.tensor_tensor(
                        pq[:].rearrange("p (k d) -> p k d", k=KH),
                        kh3,
                        q_t[:].unsqueeze(1).to_broadcast([P, KH, D]),
                        op=OP.mult,
                    )
                    nc.vector.tensor_reduce(
                        dot_q[:, ks],
                        pq[:].rearrange("p (k d) -> p k d", k=KH),
                        axis=AX.X,
                        op=OP.add,
                    )

                    pw = scrp.tile([P, KH * D], F32, tag="scr")
                    nc.gpsimd.tensor_tensor(
                        pw[:].rearrange("p (k d) -> p k d", k=KH),
                        kh3,
                        wv_t[:].unsqueeze(1).to_broadcast([P, KH, D]),
                        op=OP.mult,
                    )
                    nc.vector.tensor_reduce(
                        dot_w[:, ks],
                        pw[:].rearrange("p (k d) -> p k d", k=KH),
                        axis=AX.X,
                        op=OP.add,
                    )

                # dense write-back of keys (unchanged) while stats continue
                for h in range(2):
                    nc.scalar.dma_start(
                        kn_d[b0 : b0 + P, h * KH : (h + 1) * KH, :],
                        keys_h[h][:].rearrange("p (k d) -> p k d", k=KH),
                    )
                for h in range(2):
                    nc.scalar.dma_start(
                        vn_d[b0 : b0 + P, h * KH : (h + 1) * KH, :],
                        vals_h[h][:].rearrange("p (k d) -> p k d", k=KH),
                    )

                # ---------------- inverse norms ----------------
                scr256 = smallp.tile([P, D], F32, tag="scr256")
                qsq = smallp.tile([P, 1], F32, tag="qsq")
                nc.scalar.activation(scr256[:], q_t[:], AF.Square, accum_out=qsq[:])
                scr256b = smallp.tile([P, D], F32, tag="scr256b")
                wsq = smallp.tile([P, 1], F32, tag="wsq")
                nc.scalar.activation(scr256b[:], wv_t[:], AF.Square, accum_out=wsq[:])

                invnq = smallp.tile([P, 1], F32, tag="invnq")
                nc.scalar.activation(qsq[:], qsq[:], AF.Sqrt)
                nc.vector.tensor_scalar_add(qsq[:], qsq[:], EPS)
                nc.vector.reciprocal(invnq[:], qsq[:])

                invnw = smallp.tile([P, 1], F32, tag="invnw")
                nc.scalar.activation(wsq[:], wsq[:], AF.Sqrt)
                nc.vector.tensor_scalar_add(wsq[:], wsq[:], EPS)
                nc.vector.reciprocal(invnw[:], wsq[:])

                invnk = smallp.tile([P, K], F32, tag="invnk")
                nc.scalar.activation(normsq[:], normsq[:], AF.Sqrt)
                nc.vector.tensor_scalar_add(normsq[:], normsq[:], EPS)
                nc.vector.reciprocal(invnk[:], normsq[:])

                # ---------------- read softmax ----------------
                sim_r = smallp.tile([P, K], F32, tag="sim_r")
                nc.vector.tensor_mul(sim_r[:], dot_q[:], invnk[:])
                nc.scalar.activation(sim_r[:], sim_r[:], AF.Copy, scale=invnq[:, :1])

                # logits = sim_r + 0.5*ln(clip(strength, 0.001, 1)) - 0.02*age
                stc = smallp.tile([P, K], F32, tag="stc")
                nc.vector.tensor_scalar_max(stc[:], str_t[:], 0.001)
                nc.vector.tensor_scalar_min(stc[:], stc[:], 1.0)
                nc.scalar.activation(stc[:], stc[:], AF.Ln)
                logits = smallp.tile([P, K], F32, tag="logits")
                nc.vector.tensor_scalar(
                    logits[:], stc[:], STRENGTH_BOOST, None, op0=OP.mult
                )
                nc.vector.tensor_add(logits[:], logits[:], sim_r[:])
                agep = smallp.tile([P, K], F32, tag="agep")
                nc.vector.tensor_scalar_mul(agep[:], age_t[:], -AGE_PENALTY)
                nc.vector.tensor_add(logits[:], logits[:], agep[:])

                negmax = smallp.tile([P, 1], F32, tag="negmax")
                nc.vector.tensor_reduce(
                    negmax[:], logits[:], axis=AX.X, op=OP.max, negate=True
                )
                er = smallp.tile([P, K], F32, tag="er")
                den = smallp.tile([P, 1], F32, tag="den")
                nc.scalar.activation(
                    er[:], logits[:], AF.Exp, bias=negmax[:, :1], accum_out=den[:]
                )
                invden = smallp.tile([P, 1], F32, tag="invden")
                nc.vector.reciprocal(invden[:], den[:])
                wread = smallp.tile([P, K], F32, tag="wread")
                nc.scalar.activation(wread[:], er[:], AF.Copy, scale=invden[:, :1])

                # ---------------- write softmax / argmax ----------------
                sim_w = smallp.tile([P, K], F32, tag="sim_w")
                nc.vector.tensor_mul(sim_w[:], dot_w[:], invnk[:])
                nc.scalar.activation(sim_w[:], sim_w[:], AF.Copy, scale=invnw[:, :1])

                ln1p = smallp.tile([P, K], F32, tag="ln1p")
                nc.scalar.activation(ln1p[:], age_t[:], AF.Ln, bias=1.0)
                wlog = smallp.tile([P, K], F32, tag="wlog")
                nc.vector.tensor_scalar(
                    wlog[:], ln1p[:], EVICT_AGE_BOOST, None, op0=OP.mult
                )
                strp = smallp.tile([P, K], F32, tag="strp")
                nc.vector.tensor_scalar_mul(strp[:], str_t[:], -EVICT_STR_PENALTY)
                nc.vector.tensor_add(wlog[:], wlog[:], strp[:])
                simw50 = smallp.tile([P, K], F32, tag="simw50")
                nc.vector.tensor_scalar_mul(simw50[:], sim_w[:], WRITE_TEMP)
                nc.vector.tensor_add(wlog[:], wlog[:], simw50[:])

                negmaxw = smallp.tile([P, 1], F32, tag="negmaxw")
                nc.vector.tensor_reduce(
                    negmaxw[:], wlog[:], axis=AX.X, op=OP.max, negate=True
                )
                ew = smallp.tile([P, K], F32, tag="ew")
                denw = smallp.tile([P, 1], F32, tag="denw")
                nc.scalar.activation(
                    ew[:], wlog[:], AF.Exp, bias=negmaxw[:, :1], accum_out=denw[:]
                )
                invdenw = smallp.tile([P, 1], F32, tag="invdenw")
                nc.vector.reciprocal(invdenw[:], denw[:])
                wsoft = smallp.tile([P, K], F32, tag="wsoft")
                nc.scalar.activation(wsoft[:], ew[:], AF.Copy, scale=invdenw[:, :1])

                maxw = smallp.tile([P, 1], F32, tag="maxw")
                nc.scalar.activation(maxw[:], negmaxw[:], AF.Copy, scale=-1.0)
                oh = smallp.tile([P, K], F32, tag="oh")
                nc.vector.tensor_tensor(
                    oh[:], wlog[:], maxw[:].to_broadcast([P, K]), op=OP.is_equal
                )
                # straight-through: write_w = (hard - soft) + soft
                write_w = smallp.tile([P, K], F32, tag="write_w")
                nc.vector.tensor_sub(write_w[:], oh[:], wsoft[:])
                nc.vector.tensor_add(write_w[:], write_w[:], wsoft[:])

                # argmax index
                max8 = smallp.tile([P, 8], F32, tag="max8")
                idx8 = smallp.tile([P, 8], U32, tag="idx8")
                nc.vector.max_with_indices(max8[:], idx8[:], wlog[:])

                # ---------------- age / strength updates ----------------
                wsc = smallp.tile([P, 1], F32, tag="wsc")
                nc.vector.tensor_scalar_max(wsc[:], ws_t[:], 0.0)
                nc.vector.tensor_scalar_min(wsc[:], wsc[:], 1.0)

                a1 = smallp.tile([P, K], F32, tag="a1")
                nc.vector.tensor_scalar_add(a1[:], age_t[:], 1.0)
                omw = smallp.tile([P, K], F32, tag="omw")
                nc.scalar.activation(omw[:], write_w[:], AF.Copy, scale=-1.0, bias=1.0)
                agen = smallp.tile([P, K], F32, tag="agen")
                nc.vector.tensor_mul(agen[:], a1[:], omw[:])
                nc.sync.dma_start(an_d[b0 : b0 + P, :], agen[:])

                sd = smallp.tile([P, K], F32, tag="sd")
                nc.vector.tensor_scalar_mul(sd[:], str_t[:], STRENGTH_DECAY)
                wws = smallp.tile([P, K], F32, tag="wws")
                nc.scalar.activation(wws[:], write_w[:], AF.Copy, scale=wsc[:, :1])
                omsd = smallp.tile([P, K], F32, tag="omsd")
                nc.scalar.activation(omsd[:], sd[:], AF.Copy, scale=-1.0, bias=1.0)
                strn = smallp.tile([P, K], F32, tag="strn")
                nc.vector.tensor_mul(strn[:], wws[:], omsd[:])
                nc.vector.tensor_add(strn[:], strn[:], sd[:])
                nc.vector.tensor_scalar_max(strn[:], strn[:], 0.0)
                nc.vector.tensor_scalar_min(strn[:], strn[:], 1.0)
                nc.sync.dma_start(sn_d[b0 : b0 + P, :], strn[:])

                # ---------------- read_out via PE ----------------
                # read_out = sum_k diag(w_read[:, k]) @ vals[:, k, :]
                ro_psum = psump.tile([P, D], F32, tag="ro")
                for k in range(K):
                    dg = diagp.tile([P, P], F32, tag="diag")
                    nc.scalar.activation(
                        dg[:], ident[:], AF.Copy, scale=wread[:, k : k + 1]
                    )
                    nc.tensor.matmul(
                        ro_psum[:],
                        lhsT=dg[:],
                        rhs=vals_h[k // KH][:, (k % KH) * D : (k % KH + 1) * D],
                        start=(k == 0),
                        stop=(k == K - 1),
                    )
                ro_sb = smallp.tile([P, D], F32, tag="ro_sb")
                nc.scalar.copy(ro_sb[:], ro_psum[:])
                nc.scalar.dma_start(ro_d[b0 : b0 + P, :], ro_sb[:])

                # ---------------- scatter rows ----------------
                iota_t = smallp.tile([P, 1], I32, tag="iota")
                nc.gpsimd.iota(
                    iota_t[:], pattern=[[0, 1]], base=b0 * K, channel_multiplier=K
                )
                idx_i = smallp.tile([P, 1], I32, tag="idx_i")
                nc.vector.tensor_copy(idx_i[:], idx8[:, 0:1])
                gidx_col = gidx_all[:, t : t + 1]
                nc.vector.tensor_tensor(gidx_col, iota_t[:], idx_i[:], op=OP.add)

                ksel = rowp.tile([P, D], F32, tag="ksel")
                nc.gpsimd.indirect_dma_start(
                    out=ksel[:],
                    out_offset=None,
                    in_=keys_flat,
                    in_offset=IndirectOffsetOnAxis(ap=gidx_col, axis=0),
                )
                vsel = rowp.tile([P, D], F32, tag="vsel")
                nc.gpsimd.indirect_dma_start(
                    out=vsel[:],
                    out_offset=None,
                    in_=vals_flat,
                    in_offset=IndirectOffsetOnAxis(ap=gidx_col, axis=0),
                )

                # rate at argmax: wsoft@argmax == invdenw exactly (exp(0)=1)
                wsel = smallp.tile([P, 1], F32, tag="wsel")
                nc.scalar.activation(wsel[:], invdenw[:], AF.Copy, scale=-1.0, bias=1.0)
                nc.vector.tensor_add(wsel[:], wsel[:], invdenw[:])
                rsel = smallp.tile([P, 1], F32, tag="rsel")
                nc.vector.tensor_mul(rsel[:], wsel[:], wsc[:])
                nc.vector.tensor_scalar_mul(rsel[:], rsel[:], WRITE_ALPHA)
                omr = smallp.tile([P, 1], F32, tag="omr")
                nc.scalar.activation(omr[:], rsel[:], AF.Copy, scale=-1.0, bias=1.0)

                wkn = smallp.tile([P, D], F32, tag="wkn")
                nc.scalar.activation(wkn[:], wv_t[:], AF.Copy, scale=invnw[:, :1])

                rk = rows_k[:, t * D : (t + 1) * D]
                rv = rows_v[:, t * D : (t + 1) * D]
                ka = smallp.tile([P, D], F32, tag="ka")
                nc.scalar.activation(ka[:], ksel[:], AF.Copy, scale=omr[:, :1])
                kb = smallp.tile([P, D], F32, tag="kb")
                nc.scalar.activation(kb[:], wkn[:], AF.Copy, scale=rsel[:, :1])
                nc.vector.tensor_tensor(rk, ka[:], kb[:], op=OP.add)

                va = smallp.tile([P, D], F32, tag="va")
                nc.scalar.activation(va[:], vsel[:], AF.Copy, scale=omr[:, :1])
                vb = smallp.tile([P, D], F32, tag="vb")
                nc.scalar.activation(vb[:], wv_t[:], AF.Copy, scale=rsel[:, :1])
                nc.vector.tensor_tensor(rv, va[:], vb[:], op=OP.add)

            # ---------------- final scatters (after all dense writes) ----
            for t in range(nt):
                gidx_col = gidx_all[:, t : t + 1]
                nc.gpsimd.indirect_dma_start(
                    out=kn_flat,
                    out_offset=IndirectOffsetOnAxis(ap=gidx_col, axis=0),
                    in_=rows_k[:, t * D : (t + 1) * D],
                    in_offset=None,
                )
                nc.gpsimd.indirect_dma_start(
                    out=vn_flat,
                    out_offset=IndirectOffsetOnAxis(ap=gidx_col, axis=0),
                    in_=rows_v[:, t * D : (t + 1) * D],
                    in_offset=None,
                )

    nc.compile()
    return nc


_NC_CACHE = {}


def get_nc(bc=BC):
    if bc not in _NC_CACHE:
        _NC_CACHE[bc] = build(bc)
    return _NC_CACHE[bc]


def kernel(query_vec, write_vec, write_strength, keys, vals, age, strength):
    from concourse.bass_utils import run_bass_kernel_spmd

    nc = get_nc(BC)
    ins = {
        "query_vec": np.ascontiguousarray(query_vec, dtype=np.float32),
        "write_vec": np.ascontiguousarray(write_vec, dtype=np.float32),
        "write_strength": np.ascontiguousarray(write_strength, dtype=np.float32),
        "keys": np.ascontiguousarray(keys, dtype=np.float32),
        "vals": np.ascontiguousarray(vals, dtype=np.float32),
        "age": np.ascontiguousarray(age, dtype=np.float32),
        "strength": np.ascontiguousarray(strength, dtype=np.float32),
    }
    in_maps = [
        {k: v[i * BC : (i + 1) * BC] for k, v in ins.items()} for i in range(NCORES)
    ]
    res = run_bass_kernel_spmd(nc, in_maps, core_ids=list(range(NCORES)))
    ro = np.concatenate([res.results[i]["read_out"] for i in range(NCORES)], axis=0)
    kn = np.concatenate([res.results[i]["keys_new"] for i in range(NCORES)], axis=0)
    vn = np.concatenate([res.results[i]["vals_new"] for i in range(NCORES)], axis=0)
    an = np.concatenate([res.results[i]["age_new"] for i in range(NCORES)], axis=0)
    sn = np.concatenate([res.results[i]["str_new"] for i in range(NCORES)], axis=0)
    return (ro, (kn, vn, an, sn))


# revision 6
# speedup vs baseline: 1.0376x; 1.0376x over previous
"""Trainium2 Bass kernel for EpisodicSlotMemoryBlock.

Data-parallel over batch: B=8192 split across 8 NeuronCores (1024 each).
Per core, per 128-sample tile (phase 1):
  - keys/vals in sample-major SBUF layout; slot norms + read/write dots via
    ScalarE square + DVE segmented reduces (+ GPSIMD for one big multiply)
  - read softmax / write argmax / one-hot smalls in [128,32] tiles
    (forward write_w is exactly one-hot, so the write softmax is skipped)
  - read_out accumulated on PE: sum_k diag(w_read[:,k]) @ vals[:,k,:]
    (float32r relaxed-precision matmuls)
  - keys_new/vals_new written back densely unchanged
Phase 2 (after the loop, so the gpsimd queue never stalls mid-pipeline):
  - indirect-DMA gather of each sample's argmax slot row, blend with the
    normalized write vector, indirect-DMA scatter into keys_new/vals_new
    (DRAM WAW tracking orders scatters after the dense writes)
"""

import sys

sys.path.insert(0, "/opt/trn_rl_repo")

import numpy as np

import concourse.bacc as bacc
import concourse.bass as bass
import concourse.tile as tile
from concourse import mybir
from concourse.bass import IndirectOffsetOnAxis
from concourse.masks import make_identity

F32 = mybir.dt.float32
F32R = mybir.dt.float32r
I32 = mybir.dt.int32
U32 = mybir.dt.uint32
AF = mybir.ActivationFunctionType
OP = mybir.AluOpType
AX = mybir.AxisListType

P = 128
K = 32
D = 256
NCORES = 8
B_FULL = 8192
BC = B_FULL // NCORES

STRENGTH_DECAY = 0.995
AGE_PENALTY = 0.02
STRENGTH_BOOST = 0.5
WRITE_ALPHA = 0.5
WRITE_TEMP = 50.0
EVICT_AGE_BOOST = 0.05
EVICT_STR_PENALTY = 0.5
EPS = 1e-6


def build(bc=BC):
    nc = bacc.Bacc("TRN2", target_bir_lowering=False, debug=False)
    nt = bc // P
    KH = K // 2

    q_d = nc.dram_tensor("query_vec", [bc, D], F32, kind="ExternalInput")
    wv_d = nc.dram_tensor("write_vec", [bc, D], F32, kind="ExternalInput")
    ws_d = nc.dram_tensor("write_strength", [bc, 1], F32, kind="ExternalInput")
    keys_d = nc.dram_tensor("keys", [bc, K, D], F32, kind="ExternalInput")
    vals_d = nc.dram_tensor("vals", [bc, K, D], F32, kind="ExternalInput")
    age_d = nc.dram_tensor("age", [bc, K], F32, kind="ExternalInput")
    str_d = nc.dram_tensor("strength", [bc, K], F32, kind="ExternalInput")

    ro_d = nc.dram_tensor("read_out", [bc, D], F32, kind="ExternalOutput")
    kn_d = nc.dram_tensor("keys_new", [bc, K, D], F32, kind="ExternalOutput")
    vn_d = nc.dram_tensor("vals_new", [bc, K, D], F32, kind="ExternalOutput")
    an_d = nc.dram_tensor("age_new", [bc, K], F32, kind="ExternalOutput")
    sn_d = nc.dram_tensor("str_new", [bc, K], F32, kind="ExternalOutput")

    keys_flat = keys_d[:].rearrange("b k d -> (b k) d")
    vals_flat = vals_d[:].rearrange("b k d -> (b k) d")
    kn_flat = kn_d[:].rearrange("b k d -> (b k) d")
    vn_flat = vn_d[:].rearrange("b k d -> (b k) d")

    with tile.TileContext(nc) as tc:
        with (
            tc.tile_pool(name="keysp", bufs=2) as keysp,
            tc.tile_pool(name="valsp", bufs=2) as valsp,
            tc.tile_pool(name="scrp", bufs=3) as scrp,
            tc.tile_pool(name="smallp", bufs=2) as smallp,
            tc.tile_pool(name="rowp", bufs=2) as rowp,
            tc.tile_pool(name="persist", bufs=1) as persist,
            tc.tile_pool(name="diagp", bufs=3) as diagp,
            tc.tile_pool(name="psump", bufs=2, space="PSUM") as psump,
        ):
            # persistent across tiles (consumed by phase 2)
            rows_k = persist.tile([P, nt * D], F32)
            rows_v = persist.tile([P, nt * D], F32)
            wv_all = persist.tile([P, nt * D], F32)
            gidx_all = persist.tile([P, nt], I32)
            rsel_all = persist.tile([P, nt], F32)
            omr_all = persist.tile([P, nt], F32)
            invnw_all = persist.tile([P, nt], F32)
            ident = persist.tile([P, P], F32)
            make_identity(nc, ident[:])
            iota_t = persist.tile([P, 1], I32)
            nc.gpsimd.iota(iota_t[:], pattern=[[0, 1]], base=0, channel_multiplier=K)

            for t in range(nt):
                b0 = t * P

                # ---------------- loads ----------------
                keys_h = []
                for h in range(2):
                    kh = keysp.tile([P, KH * D], F32, tag="keysh")
                    nc.sync.dma_start(
                        kh[:].rearrange("p (k d) -> p k d", k=KH),
                        keys_d[b0 : b0 + P, h * KH : (h + 1) * KH, :],
                    )
                    keys_h.append(kh)
                vals_h = []
                for h in range(2):
                    vh = valsp.tile([P, KH * D], F32, tag="valsh")
                    nc.sync.dma_start(
                        vh[:].rearrange("p (k d) -> p k d", k=KH),
                        vals_d[b0 : b0 + P, h * KH : (h + 1) * KH, :],
                    )
                    vals_h.append(vh)

                q_t = smallp.tile([P, D], F32, tag="q")
                nc.sync.dma_start(q_t[:], q_d[b0 : b0 + P, :])
                wv_t = wv_all[:, t * D : (t + 1) * D]
                nc.sync.dma_start(wv_t, wv_d[b0 : b0 + P, :])
                ws_t = smallp.tile([P, 1], F32, tag="ws")
                nc.sync.dma_start(ws_t[:], ws_d[b0 : b0 + P, :])
                age_t = smallp.tile([P, K], F32, tag="age")
                nc.sync.dma_start(age_t[:], age_d[b0 : b0 + P, :])
                str_t = smallp.tile([P, K], F32, tag="str")
                nc.sync.dma_start(str_t[:], str_d[b0 : b0 + P, :])

                # ---------------- big stats ----------------
                normsq = smallp.tile([P, K], F32, tag="normsq")
                dot_q = smallp.tile([P, K], F32, tag="dot_q")
                dot_w = smallp.tile([P, K], F32, tag="dot_w")
                for h in range(2):
                    kh = keys_h[h]
                    kh3 = kh[:].rearrange("p (k d) -> p k d", k=KH)
                    ks = slice(h * KH, (h + 1) * KH)

                    sq = scrp.tile([P, KH * D], F32, tag="scr")
                    nc.scalar.activation(sq[:], kh[:], AF.Square)
                    nc.vector.tensor_reduce(
                        normsq[:, ks],
                        sq[:].rearrange("p (k d) -> p k d", k=KH),
                        axis=AX.X,
                        op=OP.add,
                    )

                    pq = scrp.tile([P, KH * D], F32, tag="scr")
                    nc.vector.tensor_tensor(
                        pq[:].rearrange("p (k d) -> p k d", k=KH),
                        kh3,
                        q_t[:].unsqueeze(1).to_broadcast([P, KH, D]),
                        op=OP.mult,
                    )
                    nc.vector.tensor_reduce(
                        dot_q[:, ks],
                        pq[:].rearrange("p (k d) -> p k d", k=KH),
                        axis=AX.X,
                        op=OP.add,
                    )

                    pw = scrp.tile([P, KH * D], F32, tag="scr")
                    nc.gpsimd.tensor_tensor(
                        pw[:].rearrange("p (k d) -> p k d", k=KH),
                        kh3,
                        wv_t.unsqueeze(1).to_broadcast([P, KH, D]),
                        op=OP.mult,
                    )
                    nc.vector.tensor_reduce(
                        dot_w[:, ks],
                        pw[:].rearrange("p (k d) -> p k d", k=KH),
                        axis=AX.X,
                        op=OP.add,
                    )

                # dense write-back (unchanged data) while stats continue
                for h in range(2):
                    nc.scalar.dma_start(
                        kn_d[b0 : b0 + P, h * KH : (h + 1) * KH, :],
                        keys_h[h][:].rearrange("p (k d) -> p k d", k=KH),
                    )
                    nc.scalar.dma_start(
                        vn_d[b0 : b0 + P, h * KH : (h + 1) * KH, :],
                        vals_h[h][:].rearrange("p (k d) -> p k d", k=KH),
                    )

                # ---------------- inverse norms ----------------
                scr256 = smallp.tile([P, D], F32, tag="scr256")
                qsq = smallp.tile([P, 1], F32, tag="qsq")
                nc.scalar.activation(scr256[:], q_t[:], AF.Square, accum_out=qsq[:])
                scr256b = smallp.tile([P, D], F32, tag="scr256b")
                wsq = smallp.tile([P, 1], F32, tag="wsq")
                nc.scalar.activation(scr256b[:], wv_t, AF.Square, accum_out=wsq[:])

                invnq = smallp.tile([P, 1], F32, tag="invnq")
                nc.scalar.activation(qsq[:], qsq[:], AF.Sqrt)
                nc.vector.tensor_scalar_add(qsq[:], qsq[:], EPS)
                nc.vector.reciprocal(invnq[:], qsq[:])

                invnw = smallp.tile([P, 1], F32, tag="invnw")
                nc.scalar.activation(wsq[:], wsq[:], AF.Sqrt)
                nc.vector.tensor_scalar_add(wsq[:], wsq[:], EPS)
                nc.vector.reciprocal(invnw[:], wsq[:])
                nc.vector.tensor_copy(invnw_all[:, t : t + 1], invnw[:])

                invnk = smallp.tile([P, K], F32, tag="invnk")
                nc.scalar.activation(normsq[:], normsq[:], AF.Sqrt)
                nc.vector.tensor_scalar_add(normsq[:], normsq[:], EPS)
                nc.vector.reciprocal(invnk[:], normsq[:])

                # ---------------- read softmax ----------------
                sim_r = smallp.tile([P, K], F32, tag="sim_r")
                nc.vector.tensor_mul(sim_r[:], dot_q[:], invnk[:])
                nc.scalar.activation(sim_r[:], sim_r[:], AF.Copy, scale=invnq[:, :1])

                # logits = sim_r + 0.5*ln(clip(strength, 0.001, 1)) - 0.02*age
                stc = smallp.tile([P, K], F32, tag="stc")
                nc.vector.tensor_scalar(
                    stc[:], str_t[:], 0.001, 1.0, op0=OP.max, op1=OP.min
                )
                nc.scalar.activation(stc[:], stc[:], AF.Ln)
                logits = smallp.tile([P, K], F32, tag="logits")
                nc.vector.tensor_scalar(
                    logits[:], stc[:], STRENGTH_BOOST, None, op0=OP.mult
                )
                nc.vector.tensor_add(logits[:], logits[:], sim_r[:])
                a1 = smallp.tile([P, K], F32, tag="a1")
                nc.vector.tensor_scalar_add(a1[:], age_t[:], 1.0)
                agep = smallp.tile([P, K], F32, tag="agep")
                nc.vector.tensor_scalar(
                    agep[:], a1[:], -AGE_PENALTY, AGE_PENALTY, op0=OP.mult, op1=OP.add
                )
                nc.vector.tensor_add(logits[:], logits[:], agep[:])

                negmax = smallp.tile([P, 1], F32, tag="negmax")
                nc.vector.tensor_reduce(
                    negmax[:], logits[:], axis=AX.X, op=OP.max, negate=True
                )
                er = smallp.tile([P, K], F32, tag="er")
                den = smallp.tile([P, 1], F32, tag="den")
                nc.scalar.activation(
                    er[:], logits[:], AF.Exp, bias=negmax[:, :1], accum_out=den[:]
                )
                invden = smallp.tile([P, 1], F32, tag="invden")
                nc.vector.reciprocal(invden[:], den[:])

                # ---------------- write logits / argmax one-hot ----------
                # (forward write_w == one_hot exactly; soft path not needed)
                sim_w = smallp.tile([P, K], F32, tag="sim_w")
                nc.vector.tensor_mul(sim_w[:], dot_w[:], invnk[:])
                invnw50 = smallp.tile([P, 1], F32, tag="invnw50")
                nc.vector.tensor_scalar_mul(invnw50[:], invnw[:], WRITE_TEMP)
                nc.scalar.activation(sim_w[:], sim_w[:], AF.Copy, scale=invnw50[:, :1])

                ln1p = smallp.tile([P, K], F32, tag="ln1p")
                nc.scalar.activation(ln1p[:], a1[:], AF.Ln)
                wlog = smallp.tile([P, K], F32, tag="wlog")
                nc.vector.tensor_scalar(
                    wlog[:], ln1p[:], EVICT_AGE_BOOST, None, op0=OP.mult
                )
                nc.vector.tensor_add(wlog[:], wlog[:], sim_w[:])
                strp = smallp.tile([P, K], F32, tag="strp")
                nc.vector.tensor_scalar_mul(strp[:], str_t[:], -EVICT_STR_PENALTY)
                nc.vector.tensor_add(wlog[:], wlog[:], strp[:])

                maxw = smallp.tile([P, 1], F32, tag="maxw")
                nc.vector.tensor_reduce(maxw[:], wlog[:], axis=AX.X, op=OP.max)
                noh = smallp.tile([P, K], F32, tag="noh")
                nc.vector.tensor_tensor(
                    noh[:], wlog[:], maxw[:].to_broadcast([P, K]), op=OP.not_equal
                )
                oh = smallp.tile([P, K], F32, tag="oh")
                nc.vector.tensor_scalar(
                    oh[:], noh[:], -1.0, 1.0, op0=OP.mult, op1=OP.add
                )

                max8 = smallp.tile([P, 8], F32, tag="max8")
                idx8 = smallp.tile([P, 8], U32, tag="idx8")
                nc.vector.max_with_indices(max8[:], idx8[:], wlog[:])

                # ---------------- age / strength updates ----------------
                wsc = smallp.tile([P, 1], F32, tag="wsc")
                nc.vector.tensor_scalar(
                    wsc[:], ws_t[:], 0.0, 1.0, op0=OP.max, op1=OP.min
                )
                nc.vector.tensor_scalar(
                    rsel_all[:, t : t + 1], wsc[:], WRITE_ALPHA, None, op0=OP.mult
                )
                nc.vector.tensor_scalar(
                    omr_all[:, t : t + 1],
                    wsc[:],
                    -WRITE_ALPHA,
                    1.0,
                    op0=OP.mult,
                    op1=OP.add,
                )

                agen = smallp.tile([P, K], F32, tag="agen")
                nc.vector.tensor_mul(agen[:], a1[:], noh[:])
                nc.sync.dma_start(an_d[b0 : b0 + P, :], agen[:])

                sd = smallp.tile([P, K], F32, tag="sd")
                nc.vector.tensor_scalar_mul(sd[:], str_t[:], STRENGTH_DECAY)
                wws = smallp.tile([P, K], F32, tag="wws")
                nc.scalar.activation(wws[:], oh[:], AF.Copy, scale=wsc[:, :1])
                omsd = smallp.tile([P, K], F32, tag="omsd")
                nc.vector.tensor_scalar(
                    omsd[:], sd[:], -1.0, 1.0, op0=OP.mult, op1=OP.add
                )
                strn = smallp.tile([P, K], F32, tag="strn")
                nc.vector.tensor_mul(strn[:], wws[:], omsd[:])
                nc.vector.tensor_add(strn[:], strn[:], sd[:])
                nc.vector.tensor_scalar(
                    strn[:], strn[:], 0.0, 1.0, op0=OP.max, op1=OP.min
                )
                nc.sync.dma_start(sn_d[b0 : b0 + P, :], strn[:])

                # global row index for phase-2 gather/scatter
                idx_i = smallp.tile([P, 1], I32, tag="idx_i")
                nc.vector.tensor_copy(idx_i[:], idx8[:, 0:1])
                nc.vector.tensor_scalar_add(idx_i[:], idx_i[:], b0 * K)
                nc.vector.tensor_tensor(
                    gidx_all[:, t : t + 1], iota_t[:], idx_i[:], op=OP.add
                )

                # ---------------- read_out via PE (fp32r) ----------------
                ro_psum = psump.tile([P, D], F32, tag="ro")
                for k in range(K):
                    dg = diagp.tile([P, P], F32, tag="diag")
                    nc.scalar.activation(
                        dg[:], ident[:], AF.Copy, scale=er[:, k : k + 1]
                    )
                    nc.tensor.matmul(
                        ro_psum[:],
                        lhsT=dg[:],
                        rhs=vals_h[k // KH][:, (k % KH) * D : (k % KH + 1) * D],
                        start=(k == 0),
                        stop=(k == K - 1),
                    )
                ro_sb = smallp.tile([P, D], F32, tag="ro_sb")
                nc.scalar.activation(
                    ro_sb[:], ro_psum[:], AF.Copy, scale=invden[:, :1]
                )
                nc.scalar.dma_start(ro_d[b0 : b0 + P, :], ro_sb[:])

            # ---------------- phase 2: gather / blend / scatter ----------
            for t in range(nt):
                gidx_col = gidx_all[:, t : t + 1]
                rsel_col = rsel_all[:, t : t + 1]
                omr_col = omr_all[:, t : t + 1]
                wv_t = wv_all[:, t * D : (t + 1) * D]

                ksel = rowp.tile([P, D], F32, tag="ksel")
                nc.gpsimd.indirect_dma_start(
                    out=ksel[:],
                    out_offset=None,
                    in_=keys_flat,
                    in_offset=IndirectOffsetOnAxis(ap=gidx_col, axis=0),
                )
                vsel = rowp.tile([P, D], F32, tag="vsel")
                nc.gpsimd.indirect_dma_start(
                    out=vsel[:],
                    out_offset=None,
                    in_=vals_flat,
                    in_offset=IndirectOffsetOnAxis(ap=gidx_col, axis=0),
                )

                wkn = rowp.tile([P, D], F32, tag="wkn")
                nc.scalar.activation(
                    wkn[:], wv_t, AF.Copy, scale=invnw_all[:, t : t + 1]
                )

                rk = rows_k[:, t * D : (t + 1) * D]
                rv = rows_v[:, t * D : (t + 1) * D]
                ka = rowp.tile([P, D], F32, tag="ka")
                nc.scalar.activation(ka[:], ksel[:], AF.Copy, scale=omr_col)
                kb = rowp.tile([P, D], F32, tag="kb")
                nc.scalar.activation(kb[:], wkn[:], AF.Copy, scale=rsel_col)
                nc.vector.tensor_tensor(rk, ka[:], kb[:], op=OP.add)

                va = rowp.tile([P, D], F32, tag="va")
                nc.scalar.activation(va[:], vsel[:], AF.Copy, scale=omr_col)
                vb = rowp.tile([P, D], F32, tag="vb")
                nc.scalar.activation(vb[:], wv_t, AF.Copy, scale=rsel_col)
                nc.vector.tensor_tensor(rv, va[:], vb[:], op=OP.add)

                nc.gpsimd.indirect_dma_start(
                    out=kn_flat,
                    out_offset=IndirectOffsetOnAxis(ap=gidx_col, axis=0),
                    in_=rk,
                    in_offset=None,
                )
                nc.gpsimd.indirect_dma_start(
                    out=vn_flat,
                    out_offset=IndirectOffsetOnAxis(ap=gidx_col, axis=0),
                    in_=rv,
                    in_offset=None,
                )

    nc.compile()
    return nc


_NC_CACHE = {}


def get_nc(bc=BC):
    if bc not in _NC_CACHE:
        _NC_CACHE[bc] = build(bc)
    return _NC_CACHE[bc]


def kernel(query_vec, write_vec, write_strength, keys, vals, age, strength):
    from concourse.bass_utils import run_bass_kernel_spmd

    nc = get_nc(BC)
    ins = {
        "query_vec": np.ascontiguousarray(query_vec, dtype=np.float32),
        "write_vec": np.ascontiguousarray(write_vec, dtype=np.float32),
        "write_strength": np.ascontiguousarray(write_strength, dtype=np.float32),
        "keys": np.ascontiguousarray(keys, dtype=np.float32),
        "vals": np.ascontiguousarray(vals, dtype=np.float32),
        "age": np.ascontiguousarray(age, dtype=np.float32),
        "strength": np.ascontiguousarray(strength, dtype=np.float32),
    }
    in_maps = [
        {k: v[i * BC : (i + 1) * BC] for k, v in ins.items()} for i in range(NCORES)
    ]
    res = run_bass_kernel_spmd(nc, in_maps, core_ids=list(range(NCORES)))
    ro = np.concatenate([res.results[i]["read_out"] for i in range(NCORES)], axis=0)
    kn = np.concatenate([res.results[i]["keys_new"] for i in range(NCORES)], axis=0)
    vn = np.concatenate([res.results[i]["vals_new"] for i in range(NCORES)], axis=0)
    an = np.concatenate([res.results[i]["age_new"] for i in range(NCORES)], axis=0)
    sn = np.concatenate([res.results[i]["str_new"] for i in range(NCORES)], axis=0)
    return (ro, (kn, vn, an, sn))
